# revision 5
# baseline (speedup 1.0000x reference)
"""NeuralMeshFlow Trainium2 kernel.

Strategy
--------
Shard the flattened (B=4, N=2562) = 10248 points across 8 cores: core c gets
half of batch c//2 (1281 points, padded to 1284 = 3*428).  All heavy compute
(96 MLP evals: 6 NODE blocks x 4 RK4 steps x 4 stages) runs on-device in one
SPMD NEFF; tiny conditioning math (cf vectors, AdaIN scale MLPs, initial
AdaIN) runs on host.

Device layout: activations are transposed — channels on SBUF partitions
(4 octiles x 128), points on the free dim (3 tiles x 428).  Matmuls use
out^T = lhsT.T @ rhs with lhsT = weight tiles in natural [in, out] layout and
bf16 inputs / fp32 PSUM accumulation.

RK4 is folded into the first matmul: each eval's effective input
x + sum_j C[e][j] k_j is never materialized; instead the rhs is the stacked
state tile [x; k_0..k_14] (partitions) and the lhsT is a host-precomputed
stack [W1; C[e][0] W1; ...].  The per-block combine x += sum_j D[j] k_j is a
single K=51 matmul against a striped coefficient matrix.

AdaIN means between DeformBlocks use a per-core partial sum (DVE reduce) and
a pairwise (cores 2b, 2b+1) AllReduce of 12 bytes.
"""

import numpy as np
import ml_dtypes

BF = ml_dtypes.bfloat16

B = 4
N_FULL = 2562
HALF = 1281          # points per core (2562 / 2)
P = 1284             # padded points per core (3 * 428)
TSZ = 428            # point-tile size
NT = 3               # point tiles
NBLK = 6             # NODE blocks
EV = 16              # dyn evals per block (4 RK4 steps x 4 stages)
KMAX = 3 * (1 + EV - 1) + 3   # 51 state rows: x (3) + 16 k's (48)
H = 512
TIME, N_STEPS = 0.2, 4
DT = TIME / N_STEPS

REPLICA_GROUPS = [[0, 1], [2, 3], [4, 5], [6, 7]]

TRACE = False            # set by test harness to capture an NTFF profile
LAST_RESULTS = None      # BassKernelResults of the last run (for profiling)

_CACHE = {}


def _rk4_coeffs():
    """C[e][j]: coefficient of k_j in eval e's input; Dfin[j]: coefficient in
    the final per-block combine  x_next = x + sum_j Dfin[j] k_j."""
    C = np.zeros((EV, EV), np.float64)
    Dcur = np.zeros(EV, np.float64)
    for s in range(4):
        e0 = 4 * s
        C[e0] = Dcur
        C[e0 + 1] = Dcur; C[e0 + 1][e0] = DT / 2
        C[e0 + 2] = Dcur; C[e0 + 2][e0 + 1] = DT / 2
        C[e0 + 3] = Dcur; C[e0 + 3][e0 + 2] = DT
        Dcur = Dcur.copy()
        for j, w in zip(range(e0, e0 + 4), (DT / 6, DT / 3, DT / 3, DT / 6)):
            Dcur[j] += w
    return C.astype(np.float32), Dcur.astype(np.float32)


def _build_bass():
    import concourse.bass as bass
    import concourse.tile as tile
    from concourse import bacc, mybir

    f32 = mybir.dt.float32
    bf16 = mybir.dt.bfloat16
    Alu = mybir.AluOpType
    Act = mybir.ActivationFunctionType
    ts = bass.ts

    nc = bacc.Bacc("TRN2", target_bir_lowering=False, debug=False, num_devices=8)

    x0_d = nc.dram_tensor("x0", [3, P], f32, kind="ExternalInput").ap()
    w1s_d = nc.dram_tensor("w1s", [KMAX, NBLK, EV, 4, 128], bf16, kind="ExternalInput").ap()
    w23_d = nc.dram_tensor("w23", [128, NBLK, 2, 4, 4, 128], bf16, kind="ExternalInput").ap()
    w4_d = nc.dram_tensor("w4", [128, NBLK, 4, 3], bf16, kind="ExternalInput").ap()
    b123_d = nc.dram_tensor("b123", [128, NBLK * 12], f32, kind="ExternalInput").ap()
    b4_d = nc.dram_tensor("b4", [3, NBLK], f32, kind="ExternalInput").ap()
    cf_d = nc.dram_tensor("cf", [128, NBLK * 4], f32, kind="ExternalInput").ap()
    dfin_d = nc.dram_tensor("dfin", [KMAX, 3], bf16, kind="ExternalInput").ap()
    adain_d = nc.dram_tensor("adain", [3, 9], f32, kind="ExternalInput").ap()
    out_d = nc.dram_tensor("out", [3, 3, P], f32, kind="ExternalOutput").ap()

    with tile.TileContext(nc) as tc:
        with (
            tc.tile_pool(name="consts", bufs=1) as consts,
            tc.tile_pool(name="wpool", bufs=2) as wpool,
            tc.tile_pool(name="hpool", bufs=2) as hpool,
            tc.tile_pool(name="spool", bufs=1) as spool,
            tc.tile_pool(name="pspool", bufs=2, space="PSUM") as pspool,
            tc.tile_pool(name="dpool", bufs=1, space="DRAM") as dpool,
        ):
            # ---- constants ----
            b123_sb = consts.tile([128, NBLK * 12], f32)
            nc.sync.dma_start(out=b123_sb, in_=b123_d)
            b4_sb = consts.tile([3, NBLK], f32)
            nc.sync.dma_start(out=b4_sb, in_=b4_d)
            cf_sb = consts.tile([128, NBLK * 4], f32)
            nc.sync.dma_start(out=cf_sb, in_=cf_d)
            dfin_sb = consts.tile([KMAX, 3], bf16)
            nc.sync.dma_start(out=dfin_sb, in_=dfin_d)
            adain_sb = consts.tile([3, 9], f32)
            nc.sync.dma_start(out=adain_sb, in_=adain_d)

            # ---- state ----
            x32 = spool.tile([3, P], f32)          # fp32 master of x^T
            state = spool.tile([KMAX, P], bf16)    # rows 0-2: x (bf16); rows 3+3e: k_e
            nc.sync.dma_start(out=x32, in_=x0_d)
            for t in range(NT):
                nc.gpsimd.tensor_copy(out=state[0:3, ts(t, TSZ)], in_=x32[:, ts(t, TSZ)])

            def relu_chunk(m, ps, bias_ap, out_ap):
                # out = relu(psum + bias); octiles 0-2 on ScalarE, octile 3 on
                # VectorE to balance engine load.
                if m < 3:
                    nc.scalar.activation(out=out_ap, in_=ps[:, :, :TSZ],
                                         func=Act.Relu, bias=bias_ap)
                else:
                    nc.vector.tensor_scalar(out=out_ap, in0=ps[:, :, :TSZ],
                                            scalar1=bias_ap, scalar2=0.0,
                                            op0=Alu.add, op1=Alu.max)

            for b in range(NBLK):
                w1s = wpool.tile([KMAX, EV, 4, 128], bf16, tag="w1s")
                nc.sync.dma_start(out=w1s, in_=w1s_d[:, b])
                w23 = wpool.tile([128, 2, 4, 4, 128], bf16, tag="w23")
                nc.sync.dma_start(out=w23, in_=w23_d[:, b])
                w4s = wpool.tile([128, 4, 3], bf16, tag="w4")
                nc.sync.dma_start(out=w4s, in_=w4_d[:, b])

                for e in range(EV):
                    Ke = 3 * (1 + e)
                    r1 = hpool.tile([128, 4, NT, TSZ], bf16, tag="r1")
                    h1 = hpool.tile([128, 4, NT, TSZ], bf16, tag="h1")
                    # L1: z1 = W1.T @ (x + sum C k)  via stacked lhsT
                    for m in range(4):
                        ps = pspool.tile([128, NT, 512], f32, tag="ps")
                        for t in range(NT):
                            nc.tensor.matmul(ps[:, t, :TSZ],
                                             lhsT=w1s[0:Ke, e, m, :],
                                             rhs=state[0:Ke, ts(t, TSZ)],
                                             start=True, stop=True)
                        relu_chunk(m, ps, b123_sb[:, b * 12 + m:b * 12 + m + 1], r1[:, m])
                        nc.vector.tensor_scalar(out=h1[:, m], in0=r1[:, m],
                                                scalar1=cf_sb[:, b * 4 + m:b * 4 + m + 1], scalar2=None,
                                                op0=Alu.mult)
                    # L2 / L3: h = relu(W h + b) + h
                    hprev = h1
                    for l in range(2):
                        r = hpool.tile([128, 4, NT, TSZ], bf16, tag=f"r{l+2}")
                        hn = hpool.tile([128, 4, NT, TSZ], bf16, tag=f"h{l+2}")
                        for m in range(4):
                            ps = pspool.tile([128, NT, 512], f32, tag="ps")
                            for k in range(4):
                                for t in range(NT):
                                    nc.tensor.matmul(ps[:, t, :TSZ],
                                                     lhsT=w23[:, l, k, m, :],
                                                     rhs=hprev[:, k, t, :],
                                                     start=(k == 0), stop=(k == 3))
                            relu_chunk(m, ps, b123_sb[:, b * 12 + (l + 1) * 4 + m:b * 12 + (l + 1) * 4 + m + 1], r[:, m])
                            nc.vector.tensor_tensor(out=hn[:, m], in0=r[:, m],
                                                    in1=hprev[:, m], op=Alu.add)
                        hprev = hn
                    # L4: k_e = tanh(W4.T h3 + b4) -> state rows 3+3e.
                    # Engines can't write SBUF at partition base 3+3e, so tanh
                    # lands in a base-0 scratch tile and a SBUF->SBUF DMA (which
                    # can target any partition) moves it into the state stack.
                    ps4 = pspool.tile([128, NT, 512], f32, tag="ps")
                    ktmp = hpool.tile([3, NT, TSZ], bf16, tag="ktmp")
                    for t in range(NT):
                        for k in range(4):
                            nc.tensor.matmul(ps4[0:3, t, :TSZ],
                                             lhsT=w4s[:, k, :],
                                             rhs=hprev[:, k, t, :],
                                             start=(k == 0), stop=(k == 3))
                        nc.scalar.activation(out=ktmp[:, t, :],
                                             in_=ps4[0:3, t, :TSZ],
                                             func=Act.Tanh, bias=b4_sb[:, b:b + 1])
                        nc.sync.dma_start(out=state[3 + 3 * e:6 + 3 * e, ts(t, TSZ)],
                                          in_=ktmp[:, t, :])

                # block combine: x += sum_j Dfin[j] k_j  (K=51 matmul)
                psc = pspool.tile([128, NT, 512], f32, tag="ps")
                for t in range(NT):
                    nc.tensor.matmul(psc[0:3, t, :TSZ], lhsT=dfin_sb[:, :],
                                     rhs=state[:, ts(t, TSZ)], start=True, stop=True)
                for t in range(NT):
                    nc.vector.tensor_tensor(out=x32[:, ts(t, TSZ)], in0=x32[:, ts(t, TSZ)],
                                            in1=psc[0:3, t, :TSZ], op=Alu.add)

                if b % 2 == 1:
                    # AdaIN after each DeformBlock: x = A + M*x - M*mean(x)
                    jj = (b - 1) // 2
                    sums = spool.tile([3, 1], f32, tag="sums")
                    tot = spool.tile([3, 1], f32, tag="tot")
                    tmp = spool.tile([3, 1], f32, tag="tmp")
                    shift = spool.tile([3, 1], f32, tag="shift")
                    nc.vector.reduce_sum(out=sums, in_=x32[:, 0:HALF],
                                         axis=mybir.AxisListType.X)
                    cc_in = dpool.tile([3, 1], f32, tag=f"cc_in{jj}")
                    cc_out = dpool.tile([3, 1], f32, tag=f"cc_out{jj}")
                    nc.sync.dma_start(out=cc_in, in_=sums)
                    nc.gpsimd.collective_compute(
                        "AllReduce", Alu.add, replica_groups=REPLICA_GROUPS,
                        ins=[cc_in.opt()], outs=[cc_out.opt()])
                    nc.sync.dma_start(out=tot, in_=cc_out)
                    # shift = A - (M/N) * total ; x = M*x + shift
                    nc.gpsimd.tensor_tensor(out=tmp, in0=tot,
                                            in1=adain_sb[:, 3 * jj + 2:3 * jj + 3], op=Alu.mult)
                    nc.gpsimd.tensor_tensor(out=shift, in0=adain_sb[:, 3 * jj + 1:3 * jj + 2],
                                            in1=tmp, op=Alu.subtract)
                    for t in range(NT):
                        nc.vector.tensor_scalar(out=x32[:, ts(t, TSZ)],
                                                in0=x32[:, ts(t, TSZ)],
                                                scalar1=adain_sb[:, 3 * jj:3 * jj + 1],
                                                scalar2=shift,
                                                op0=Alu.mult, op1=Alu.add)
                        if b < NBLK - 1:
                            nc.gpsimd.tensor_copy(out=state[0:3, ts(t, TSZ)],
                                                  in_=x32[:, ts(t, TSZ)])
                    nc.sync.dma_start(out=out_d[jj], in_=x32[:, :])
                else:
                    for t in range(NT):
                        nc.gpsimd.tensor_copy(out=state[0:3, ts(t, TSZ)],
                                              in_=x32[:, ts(t, TSZ)])

    nc.compile()
    return nc


def _host_prep(inputs):
    """Host-side preprocessing: shared weights + per-core tensors."""
    clv = np.asarray(inputs["content_latent_vector"], np.float32)   # (B,1,512)
    ap = np.asarray(inputs["adain_params"], np.float32)             # (B,24)
    verts = np.asarray(inputs["vertices"], np.float32)              # (N,3)
    W1 = np.asarray(inputs["W1"], np.float32)
    W2 = np.asarray(inputs["W2"], np.float32)
    W3 = np.asarray(inputs["W3"], np.float32)
    W4 = np.asarray(inputs["W4"], np.float32)
    b1 = np.asarray(inputs["b1"], np.float32)
    b2 = np.asarray(inputs["b2"], np.float32)
    b3 = np.asarray(inputs["b3"], np.float32)
    b4 = np.asarray(inputs["b4"], np.float32)
    Wc = np.asarray(inputs["Wc"], np.float32)
    bc = np.asarray(inputs["bc"], np.float32)
    Wn1 = np.asarray(inputs["Wn1"], np.float32)
    bn1 = np.asarray(inputs["bn1"], np.float32)
    Wn2 = np.asarray(inputs["Wn2"], np.float32)
    bn2 = np.asarray(inputs["bn2"], np.float32)

    C, Dfin = _rk4_coeffs()

    # shared weight packs
    w1s = np.zeros((NBLK, EV, KMAX, H), np.float32)
    for b in range(NBLK):
        for e in range(EV):
            w1s[b, e, 0:3] = W1[b]
            for j in range(e):
                if C[e][j] != 0.0:
                    w1s[b, e, 3 + 3 * j:6 + 3 * j] = C[e][j] * W1[b]
    w1s = (w1s.reshape(NBLK, EV, KMAX, 4, 128)
              .transpose(2, 0, 1, 3, 4)).astype(BF)          # [51,6,16,4,128]

    w23 = (np.stack([W2, W3], 1)
             .reshape(NBLK, 2, 4, 128, 4, 128)
             .transpose(3, 0, 1, 2, 4, 5)).astype(BF)        # [128,6,2,4,4,128]
    w4 = W4.reshape(NBLK, 4, 128, 3).transpose(2, 0, 1, 3).astype(BF)  # [128,6,4,3]
    b123 = (np.stack([b1, b2, b3], 1)
              .reshape(NBLK, 3, 4, 128)
              .transpose(3, 0, 1, 2)
              .reshape(128, NBLK * 12)).astype(np.float32).copy()      # [128, 6*3*4]
    b4p = b4.T.astype(np.float32).copy()                               # [3,6]
    dfin = np.zeros((KMAX, 3), np.float32)
    for j in range(EV):
        for i in range(3):
            dfin[3 + 3 * j + i, i] = Dfin[j]
    dfin = dfin.astype(BF)

    def sigmoid(x):
        return 1.0 / (1.0 + np.exp(-x))

    # conditioning features per block: (6, B, 512)
    cf_all = np.stack([np.tanh(clv @ Wc[k] + bc[k])[:, 0, :] for k in range(NBLK)])

    # AdaIN affine constants per j (including initial j=0 applied on host)
    adain_M = np.zeros((4, B, 3), np.float32)
    adain_A = np.zeros((4, B, 3), np.float32)
    for j in range(4):
        p6 = ap[:, 6 * j:6 * j + 6]
        scale = sigmoid(np.maximum(clv @ Wn1[j] + bn1[j], 0.0) @ Wn2[j] + bn2[j])[:, 0, :]
        adain_M[j] = p6[:, 3:] * (1.0 - scale)
        adain_A[j] = p6[:, :3]

    # initial AdaIN on host: x0 = A0 + M0*(verts - mean(verts)) per batch
    vmean = verts.mean(0)
    x0_full = (adain_A[0][:, None, :]
               + adain_M[0][:, None, :] * (verts[None] - vmean[None, None]))  # (B,N,3)

    shared = {"w1s": w1s, "w23": w23, "w4": w4, "b123": b123, "b4": b4p,
              "dfin": dfin}

    in_maps = []
    for c in range(8):
        bidx, half = c // 2, c % 2
        xc = np.zeros((3, P), np.float32)
        xc[:, :HALF] = x0_full[bidx, half * HALF:(half + 1) * HALF].T
        cfc = cf_all[:, bidx, :].reshape(NBLK, 4, 128).transpose(2, 0, 1).reshape(128, NBLK * 4)
        adain_c = np.zeros((3, 9), np.float32)
        for j in range(1, 4):
            adain_c[:, 3 * (j - 1) + 0] = adain_M[j][bidx]
            adain_c[:, 3 * (j - 1) + 1] = adain_A[j][bidx]
            adain_c[:, 3 * (j - 1) + 2] = adain_M[j][bidx] / np.float32(N_FULL)
        m = dict(shared)
        m["x0"] = xc
        m["cf"] = np.ascontiguousarray(cfc.astype(np.float32))
        m["adain"] = adain_c
        in_maps.append(m)
    return in_maps


def kernel(**inputs) -> np.ndarray:
    global LAST_RESULTS
    from concourse.bass_utils import run_bass_kernel_spmd

    if "nc" not in _CACHE:
        _CACHE["nc"] = _build_bass()
    nc = _CACHE["nc"]

    in_maps = _host_prep(inputs)
    res = run_bass_kernel_spmd(nc, in_maps, core_ids=list(range(8)), trace=TRACE)
    LAST_RESULTS = res

    full = np.zeros((3, B, N_FULL, 3), np.float32)
    for c in range(8):
        bidx, half = c // 2, c % 2
        chunk = res.results[c]["out"][:, :, :HALF]          # (3, 3ch, HALF)
        full[:, bidx, half * HALF:(half + 1) * HALF, :] = chunk.transpose(0, 2, 1)
    return full
